# revision 25
# baseline (speedup 1.0000x reference)
"""Trainium2 Bass kernel for nn_Inter_RM_6940667150684 (gnn_message_passing).

Math (per example n):
  g[n,m,:] = relu(f[n,m,:] @ W[m].T)            (W[m,e,d], contract d)
  s[n,j,k] = ||g_j - g_k||^2
  edges    = tanh(sqrt(relu(s)))  (diag 0, symmetric)
  y[n]     = 0.5*sum_m f[n,m,:] + sum_k c_k[n]*g[n,k,:]
  c_k[n]   = 0.5*sum_{j!=k} tanh(sqrt(relu(s_jk)))

Sharding: pure data parallel over batch N=8192 -> 8 cores x 1024 rows.

v3 design (engine-balanced, ~4.6 us/ntile target):
  - f pre-transposed on host (fT[d,m,n]) -> no PE transposes; g and gT both
    come straight from matmuls (18 MMs/ntile).
  - pairwise products on DVE as 9 shift-slab bf16 tensor_muls (h[d,t,n]).
  - s = q_j + q_k - 2*dots computed in ONE fused PE stage: 45 accumulating
    matmuls with rank-1 selector weights (ones_d x coef_t), accumulating
    s[36,128] directly in PSUM (no dots materialization, no extra copies).
  - relu copies on ACT; sqrt/tanh on ACT; s-relu on DVE.
  - c via single PE matmul e.T @ minc -> cT[n,k] fp32.
  - combine: 9 tensor_scalar muls on DVE (4x mode) + add-tree on GPSIMD.
  - hs = sum_m 0.5*f on GPSIMD tree (0.5 folded into host-side f copy).
"""

import sys

sys.path.insert(0, "/opt/trn_rl_repo")

import numpy as np

N, M, D, E = 8192, 9, 128, 128
NCORES = 8
NLOC = N // NCORES          # 1024 rows per core
NT = NLOC // 128            # 8 ntiles of 128 examples
NPAIR = 45                  # sigma-slab tiles: 9 self + 36 cross
NP2 = 36                    # strict pairs j<k

# sigma-slab ordering: t in [0,9) -> self slice i=t (g_i * g_i);
# then for sigma=1..8, slices i=0..8-sigma -> pair (i, i+sigma).
_SLABS = [(0, i) for i in range(M)] + [
    (s, i) for s in range(1, M) for i in range(M - s)
]


def _pair_idx(j, k):
    # lex index of pair (j,k), j<k, in 36-vector
    return j * (2 * M - j - 1) // 2 + (k - j - 1)


def _host_consts():
    # Composed reduce weights: s[p,n] = sum_t coef[t,p] * sum_d h[d,t,n]
    # Weight tile for slab t is ones_d outer coef_t -> [128, 45, 36].
    sw = np.zeros((NPAIR, NP2), np.float32)
    for t, (s, i) in enumerate(_SLABS):
        if s == 0:
            for k in range(M):
                if k != i:
                    sw[t, _pair_idx(min(i, k), max(i, k))] += 1.0
        else:
            sw[t, _pair_idx(i, i + s)] = -2.0
    sw_full = np.broadcast_to(sw[None, :, :], (D, NPAIR, NP2))
    # minc: c_k = 0.5 * sum_{pairs p containing k} e_p  -> [36, 9]
    minc = np.zeros((NP2, M), np.float32)
    for j in range(M):
        for k in range(j + 1, M):
            p = _pair_idx(j, k)
            minc[p, j] = 0.5
            minc[p, k] = 0.5
    return np.ascontiguousarray(sw_full.reshape(D, NPAIR * NP2)), minc


def _emit(nc, reps=1):
    from concourse import bass, tile

    mybir = bass.mybir
    FP32 = mybir.dt.float32
    BF16 = mybir.dt.bfloat16
    AF = mybir.ActivationFunctionType

    ft_dr = nc.dram_tensor("ft", [D, NT * M * 128], BF16, kind="ExternalInput")
    fh_dr = nc.dram_tensor("fh", [NLOC, M * D], BF16, kind="ExternalInput")
    wt_dr = nc.dram_tensor("wt", [D, M * E], BF16, kind="ExternalInput")
    sw_dr = nc.dram_tensor("sw", [D, NPAIR * NP2], BF16, kind="ExternalInput")
    minc_dr = nc.dram_tensor("minc", [NP2, M], BF16, kind="ExternalInput")
    y_dr = nc.dram_tensor("y", [NLOC, E], FP32, kind="ExternalOutput")

    GRPS = [(0, 4), (4, 8), (8, 9)]

    with tile.TileContext(nc) as tc:
        with (
            tc.tile_pool(name="const", bufs=1) as cpool,
            tc.tile_pool(name="fin", bufs=8) as fpool,
            tc.tile_pool(name="gbuf", bufs=3) as gpool,
            tc.tile_pool(name="hbuf", bufs=3) as hpool,
            tc.tile_pool(name="work", bufs=2) as wpool,
            tc.tile_pool(name="ps_g", bufs=2, space=bass.MemorySpace.PSUM) as ps_g,
            tc.tile_pool(name="ps_t", bufs=2, space=bass.MemorySpace.PSUM) as ps_t,
            tc.tile_pool(name="ps_s", bufs=2, space=bass.MemorySpace.PSUM) as ps_s,
        ):
            # ---- constants ----
            wt_sb = cpool.tile([D, M * E], BF16, tag="wt")
            sw_sb = cpool.tile([D, NPAIR, NP2], BF16, tag="sw")
            minc_sb = cpool.tile([NP2, M], BF16, tag="minc")
            nc.sync.dma_start(wt_sb[:], wt_dr[:])
            nc.sync.dma_start(sw_sb[:].rearrange("p a b -> p (a b)"), sw_dr[:])
            nc.sync.dma_start(minc_sb[:], minc_dr[:])

            def _body():
                for nt in range(NT):
                    ft_sb = fpool.tile([D, M, 128], BF16, tag="ft")
                    fh_sb = fpool.tile([128, M, D], BF16, tag="fh")
                    nc.sync.dma_start(
                        ft_sb[:].rearrange("p a b -> p (a b)"),
                        ft_dr[:, nt * M * 128:(nt + 1) * M * 128],
                    )
                    nc.sync.dma_start(
                        fh_sb[:].rearrange("p a b -> p (a b)"),
                        fh_dr[nt * 128:(nt + 1) * 128, :],
                    )

                    g_sb = gpool.tile([128, M, E], BF16, tag="g")
                    gT_sb = gpool.tile([E, M, 128], BF16, tag="gT")

                    # gT first: it feeds the products chain (longest path);
                    # g is only needed at the combine tail.
                    for a, b in GRPS:
                        gt_ps = ps_t.tile([128, 4, 128], FP32, tag="gt")
                        for i in range(b - a):
                            m = a + i
                            nc.tensor.matmul(
                                gt_ps[:, i, :], wt_sb[:, m * E:(m + 1) * E],
                                ft_sb[:, m, :],
                            )
                        nc.scalar.activation(
                            gT_sb[:, a:b, :], gt_ps[:, 0:b - a, :], AF.Relu
                        )
                    for a, b in GRPS:
                        g_ps = ps_g.tile([128, 4, 128], FP32, tag="g")
                        for i in range(b - a):
                            m = a + i
                            nc.tensor.matmul(
                                g_ps[:, i, :], ft_sb[:, m, :],
                                wt_sb[:, m * E:(m + 1) * E],
                            )
                        nc.scalar.activation(
                            g_sb[:, a:b, :], g_ps[:, 0:b - a, :], AF.Relu
                        )

                    # hs = sum_m (0.5*f)  (GPSIMD tree; 0.5 folded on host)
                    t4 = wpool.tile([128, 4, D], BF16, tag="t4")
                    t2 = wpool.tile([128, 2, D], BF16, tag="t2")
                    t1 = wpool.tile([128, D], BF16, tag="t1")
                    hs = wpool.tile([128, D], BF16, tag="hs")
                    nc.gpsimd.tensor_add(t4[:], fh_sb[:, 0:4, :], fh_sb[:, 4:8, :])
                    nc.gpsimd.tensor_add(t2[:], t4[:, 0:2, :], t4[:, 2:4, :])
                    nc.gpsimd.tensor_add(t1[:], t2[:, 0, :], t2[:, 1, :])
                    nc.gpsimd.tensor_add(hs[:], t1[:], fh_sb[:, 8, :])

                    # pairwise products h[d,t,n] (DVE, bf16 2x), with the
                    # fused-reduce selector MMs interleaved per slab so PE
                    # starts reducing while DVE still produces later slabs.
                    h_sb = hpool.tile([D, NPAIR, 128], BF16, tag="h")
                    s_ps = ps_s.tile([128, 128], FP32, tag="s")
                    off = 0
                    for s in range(M):
                        w = M - s
                        if s == 0:
                            nc.vector.tensor_mul(
                                h_sb[:, 0:M, :], gT_sb[:, 0:M, :], gT_sb[:, 0:M, :]
                            )
                        else:
                            nc.vector.tensor_mul(
                                h_sb[:, off:off + w, :],
                                gT_sb[:, 0:w, :], gT_sb[:, s:M, :],
                            )
                        for t in range(off, off + w):
                            nc.tensor.matmul(
                                s_ps[0:NP2, :], sw_sb[:, t, :], h_sb[:, t, :],
                                start=(t == 0), stop=(t == NPAIR - 1),
                            )
                        off += w

                    # e = tanh(sqrt(s)) via single-table Ln/Exp path:
                    #   r = Exp(0.5*Ln(max(s,tiny))); v = Exp(2*r)
                    #   e = 1 - 2/(v+1)   (reciprocal approx on DVE)
                    s0 = wpool.tile([128, 128], BF16, tag="s0")
                    lns = wpool.tile([128, 128], BF16, tag="lns")
                    r_sb = wpool.tile([128, 128], BF16, tag="r")
                    v_sb = wpool.tile([128, 128], FP32, tag="v")
                    w_sb = wpool.tile([128, 128], FP32, tag="w")
                    rec = wpool.tile([128, 128], FP32, tag="rec")
                    e_sb = wpool.tile([128, 128], BF16, tag="e")
                    nc.vector.tensor_scalar_max(s0[0:NP2, :], s_ps[0:NP2, :], 1e-30)
                    nc.scalar.activation(lns[0:NP2, :], s0[0:NP2, :], AF.Ln)
                    nc.scalar.activation(
                        r_sb[0:NP2, :], lns[0:NP2, :], AF.Exp, scale=0.5
                    )
                    nc.scalar.activation(
                        v_sb[0:NP2, :], r_sb[0:NP2, :], AF.Exp, scale=2.0
                    )
                    nc.vector.tensor_scalar_add(w_sb[0:NP2, :], v_sb[0:NP2, :], 1.0)
                    nc.vector.reciprocal_approx_fast(rec[0:NP2, :], w_sb[0:NP2, :])
                    nc.vector.tensor_scalar(
                        e_sb[0:NP2, :], rec[0:NP2, :],
                        -2.0, 1.0,
                        bass.mybir.AluOpType.mult, bass.mybir.AluOpType.add,
                    )

                    # cT[n,k] = sum_p e[p,n] * minc[p,k]  (PE), fp32
                    ct_ps = ps_s.tile([128, 16], FP32, tag="ct")
                    nc.tensor.matmul(
                        ct_ps[0:128, 0:M], e_sb[0:NP2, :], minc_sb[0:NP2, 0:M]
                    )
                    cT_sb = wpool.tile([128, 16], FP32, tag="cT")
                    nc.scalar.activation(cT_sb[:, 0:M], ct_ps[0:128, 0:M], AF.Copy)

                    # combine: tk = c_k * g_k (DVE 4x), tree-sum on GPSIMD
                    tk = wpool.tile([128, M, E], BF16, tag="tk")
                    for k in range(M):
                        nc.vector.tensor_scalar_mul(
                            tk[:, k, :], g_sb[:, k, :], cT_sb[:, k:k + 1]
                        )
                    u4 = wpool.tile([128, 4, E], BF16, tag="u4")
                    u2 = wpool.tile([128, 2, E], BF16, tag="u2")
                    u1 = wpool.tile([128, E], BF16, tag="u1")
                    y1 = wpool.tile([128, E], BF16, tag="y1")
                    y_sb = wpool.tile([128, E], FP32, tag="y")
                    nc.vector.tensor_add(u4[:], tk[:, 0:4, :], tk[:, 4:8, :])
                    nc.vector.tensor_add(u2[:], u4[:, 0:2, :], u4[:, 2:4, :])
                    nc.vector.tensor_add(u1[:], u2[:, 0, :], u2[:, 1, :])
                    nc.vector.tensor_add(y1[:], u1[:], tk[:, 8, :])
                    nc.vector.tensor_add(y_sb[:], y1[:], hs[:])
                    nc.sync.dma_start(y_dr[nt * 128:(nt + 1) * 128, :], y_sb[:])

            for _ in range(reps):
                _body()


def _build_nc(reps=1):
    from concourse import bacc, mybir
    from concourse.hw_specs import get_activation_tables

    nc = bacc.Bacc(target_bir_lowering=False, debug=False)
    _emit(nc, reps=reps)

    # All activation funcs used (Relu/Ln/Exp/Copy) live in one table set;
    # replace the per-func first-match load pass (which thrashes between
    # sets) with a single load of that set at kernel start.
    tables = list(get_activation_tables(nc.m.arch))
    set_id = tables.index("natural_log_exp_and_others")
    funcs = {
        i.func
        for b in nc.main_func.blocks
        for i in b.instructions
        if isinstance(i, mybir.InstActivation)
    }
    allowed = get_activation_tables(nc.m.arch)["natural_log_exp_and_others"]
    assert funcs <= allowed | {mybir.ActivationFunctionType.Copy}, funcs

    def _single_act_load():
        inst = mybir.InstLoadActFuncSet(
            name=nc.get_next_instruction_name(),
            act_func_set_id=set_id,
            engine=mybir.EngineType.Activation,
        )
        nc.main_func.blocks[0].instructions.insert(0, inst)

    nc.insert_act_table_loads = _single_act_load
    nc.compile()
    return nc


def _prepare(f: np.ndarray, W: np.ndarray, reps=1):
    import ml_dtypes

    BF = ml_dtypes.bfloat16
    f = np.asarray(f, np.float32)
    wt = np.ascontiguousarray(
        np.transpose(np.asarray(W, np.float32), (2, 0, 1)).reshape(D, M * E).astype(BF)
    )
    sw, minc = _host_consts()
    base = {
        "wt": wt,
        "sw": np.ascontiguousarray(sw.astype(BF)),
        "minc": np.ascontiguousarray(minc.astype(BF)),
    }

    nc = _build_nc(reps=reps)
    in_maps = []
    for c in range(NCORES):
        fc = f[c * NLOC:(c + 1) * NLOC]                      # [1024, 9, 128]
        ft = np.transpose(fc.reshape(NT, 128, M, D), (3, 0, 2, 1))
        ft = np.ascontiguousarray(ft.reshape(D, NT * M * 128).astype(BF))
        fh = np.ascontiguousarray((0.5 * fc).reshape(NLOC, M * D).astype(BF))
        in_maps.append(dict(base, ft=ft, fh=fh))
    return nc, in_maps


def _run(f: np.ndarray, W: np.ndarray, trace: bool = False):
    from concourse.bass_utils import run_bass_kernel_spmd

    nc, in_maps = _prepare(f, W)
    res = run_bass_kernel_spmd(nc, in_maps, list(range(NCORES)), trace=trace)
    out = np.concatenate([np.asarray(r["y"]) for r in res.results], axis=0)
    return np.ascontiguousarray(out.astype(np.float32)), res


def kernel(f: np.ndarray, W: np.ndarray) -> np.ndarray:
    out, _ = _run(f, W, trace=False)
    return out


if __name__ == "__main__":
    rng = np.random.default_rng(0)
    f = rng.standard_normal((N, M, D), dtype=np.float32)
    W = rng.standard_normal((M, E, D), dtype=np.float32)
    y = kernel(f=f, W=W)
    print("kernel out", y.shape, y.dtype, float(np.abs(y).mean()))


# revision 27
# speedup vs baseline: 1.2286x; 1.2286x over previous
"""Trainium2 Bass kernel for nn_Inter_RM_6940667150684 (gnn_message_passing).

Math (per example n):
  g[n,m,:] = relu(f[n,m,:] @ W[m].T)            (W[m,e,d], contract d)
  s[n,j,k] = ||g_j - g_k||^2
  edges    = tanh(sqrt(relu(s)))  (diag 0, symmetric)
  y[n]     = 0.5*sum_m f[n,m,:] + sum_k c_k[n]*g[n,k,:]
  c_k[n]   = 0.5*sum_{j!=k} tanh(sqrt(relu(s_jk)))

Sharding: pure data parallel over batch N=8192 -> 8 cores x 1024 rows.

v3 design (engine-balanced, ~4.6 us/ntile target):
  - f pre-transposed on host (fT[d,m,n]) -> no PE transposes; g and gT both
    come straight from matmuls (18 MMs/ntile).
  - pairwise products on DVE as 9 shift-slab bf16 tensor_muls (h[d,t,n]).
  - s = q_j + q_k - 2*dots computed in ONE fused PE stage: 45 accumulating
    matmuls with rank-1 selector weights (ones_d x coef_t), accumulating
    s[36,128] directly in PSUM (no dots materialization, no extra copies).
  - relu copies on ACT; sqrt/tanh on ACT; s-relu on DVE.
  - c via single PE matmul e.T @ minc -> cT[n,k] fp32.
  - combine: 9 tensor_scalar muls on DVE (4x mode) + add-tree on GPSIMD.
  - hs = sum_m 0.5*f on GPSIMD tree (0.5 folded into host-side f copy).
"""

import sys

sys.path.insert(0, "/opt/trn_rl_repo")

import numpy as np

N, M, D, E = 8192, 9, 128, 128
NCORES = 8
NLOC = N // NCORES          # 1024 rows per core
NT = NLOC // 128            # 8 ntiles of 128 examples
NPAIR = 45                  # sigma-slab tiles: 9 self + 36 cross
NP2 = 36                    # strict pairs j<k

# sigma-slab ordering: t in [0,9) -> self slice i=t (g_i * g_i);
# then for sigma=1..8, slices i=0..8-sigma -> pair (i, i+sigma).
_SLABS = [(0, i) for i in range(M)] + [
    (s, i) for s in range(1, M) for i in range(M - s)
]


def _pair_idx(j, k):
    # lex index of pair (j,k), j<k, in 36-vector
    return j * (2 * M - j - 1) // 2 + (k - j - 1)


def _host_consts():
    # Composed reduce weights: s[p,n] = sum_t coef[t,p] * sum_d h[d,t,n]
    # Weight tile for slab t is ones_d outer coef_t -> [128, 45, 36].
    sw = np.zeros((NPAIR, NP2), np.float32)
    for t, (s, i) in enumerate(_SLABS):
        if s == 0:
            for k in range(M):
                if k != i:
                    sw[t, _pair_idx(min(i, k), max(i, k))] += 1.0
        else:
            sw[t, _pair_idx(i, i + s)] = -2.0
    sw_full = np.broadcast_to(sw[None, :, :], (D, NPAIR, NP2))
    # minc: c_k = 0.5 * sum_{pairs p containing k} e_p  -> [36, 9]
    minc = np.zeros((NP2, M), np.float32)
    for j in range(M):
        for k in range(j + 1, M):
            p = _pair_idx(j, k)
            minc[p, j] = 0.5
            minc[p, k] = 0.5
    return np.ascontiguousarray(sw_full.reshape(D, NPAIR * NP2)), minc


def _emit(nc, reps=1):
    from concourse import bass, tile

    mybir = bass.mybir
    FP32 = mybir.dt.float32
    BF16 = mybir.dt.bfloat16
    AF = mybir.ActivationFunctionType

    ft_dr = nc.dram_tensor("ft", [D, NT * M * 128], BF16, kind="ExternalInput")
    fh_dr = nc.dram_tensor("fh", [NLOC, M * D], BF16, kind="ExternalInput")
    wt_dr = nc.dram_tensor("wt", [D, M * E], BF16, kind="ExternalInput")
    sw_dr = nc.dram_tensor("sw", [D, NPAIR * NP2], BF16, kind="ExternalInput")
    minc_dr = nc.dram_tensor("minc", [NP2, M], BF16, kind="ExternalInput")
    y_dr = nc.dram_tensor("y", [NLOC, E], FP32, kind="ExternalOutput")

    GRPS = [(0, 4), (4, 8), (8, 9)]

    with tile.TileContext(nc) as tc:
        with (
            tc.tile_pool(name="const", bufs=1) as cpool,
            tc.tile_pool(name="fin", bufs=6) as fpool,
            tc.tile_pool(name="gbuf", bufs=3) as gpool,
            tc.tile_pool(name="hbuf", bufs=4) as hpool,
            tc.tile_pool(name="work", bufs=2) as wpool,
            tc.tile_pool(name="ps_g", bufs=2, space=bass.MemorySpace.PSUM) as ps_g,
            tc.tile_pool(name="ps_t", bufs=2, space=bass.MemorySpace.PSUM) as ps_t,
            tc.tile_pool(name="ps_s", bufs=2, space=bass.MemorySpace.PSUM) as ps_s,
        ):
            # ---- constants ----
            wt_sb = cpool.tile([D, M * E], BF16, tag="wt")
            sw_sb = cpool.tile([D, NPAIR, NP2], BF16, tag="sw")
            minc_sb = cpool.tile([NP2, M], BF16, tag="minc")
            nc.sync.dma_start(wt_sb[:], wt_dr[:])
            nc.sync.dma_start(sw_sb[:].rearrange("p a b -> p (a b)"), sw_dr[:])
            nc.sync.dma_start(minc_sb[:], minc_dr[:])

            def _body():
                for nt in range(NT):
                    ft_sb = fpool.tile([D, M, 128], BF16, tag="ft")
                    fh_sb = fpool.tile([128, M, D], BF16, tag="fh")
                    nc.sync.dma_start(
                        ft_sb[:].rearrange("p a b -> p (a b)"),
                        ft_dr[:, nt * M * 128:(nt + 1) * M * 128],
                    )
                    nc.sync.dma_start(
                        fh_sb[:].rearrange("p a b -> p (a b)"),
                        fh_dr[nt * 128:(nt + 1) * 128, :],
                    )

                    g_sb = gpool.tile([128, M, E], BF16, tag="g")
                    gT_sb = gpool.tile([E, M, 128], BF16, tag="gT")

                    # gT first: it feeds the products chain (longest path);
                    # g is only needed at the combine tail.
                    for a, b in GRPS:
                        gt_ps = ps_t.tile([128, 4, 128], FP32, tag="gt")
                        for i in range(b - a):
                            m = a + i
                            nc.tensor.matmul(
                                gt_ps[:, i, :], wt_sb[:, m * E:(m + 1) * E],
                                ft_sb[:, m, :],
                            )
                        nc.scalar.activation(
                            gT_sb[:, a:b, :], gt_ps[:, 0:b - a, :], AF.Relu
                        )
                    for a, b in GRPS:
                        g_ps = ps_g.tile([128, 4, 128], FP32, tag="g")
                        for i in range(b - a):
                            m = a + i
                            nc.tensor.matmul(
                                g_ps[:, i, :], ft_sb[:, m, :],
                                wt_sb[:, m * E:(m + 1) * E],
                            )
                        nc.scalar.activation(
                            g_sb[:, a:b, :], g_ps[:, 0:b - a, :], AF.Relu
                        )

                    # hs = sum_m (0.5*f)  (GPSIMD tree; 0.5 folded on host)
                    t4 = wpool.tile([128, 4, D], BF16, tag="t4")
                    t2 = wpool.tile([128, 2, D], BF16, tag="t2")
                    t1 = wpool.tile([128, D], BF16, tag="t1")
                    hs = wpool.tile([128, D], BF16, tag="hs")
                    nc.gpsimd.tensor_add(t4[:], fh_sb[:, 0:4, :], fh_sb[:, 4:8, :])
                    nc.gpsimd.tensor_add(t2[:], t4[:, 0:2, :], t4[:, 2:4, :])
                    nc.gpsimd.tensor_add(t1[:], t2[:, 0, :], t2[:, 1, :])
                    nc.gpsimd.tensor_add(hs[:], t1[:], fh_sb[:, 8, :])

                    # pairwise products h[d,t,n] (DVE, bf16 2x), with the
                    # fused-reduce selector MMs interleaved per slab so PE
                    # starts reducing while DVE still produces later slabs.
                    h_sb = hpool.tile([D, NPAIR, 128], BF16, tag="h")
                    s_ps = ps_s.tile([128, 128], FP32, tag="s")
                    off = 0
                    for s in range(M):
                        w = M - s
                        if s == 0:
                            nc.vector.tensor_mul(
                                h_sb[:, 0:M, :], gT_sb[:, 0:M, :], gT_sb[:, 0:M, :]
                            )
                        else:
                            nc.vector.tensor_mul(
                                h_sb[:, off:off + w, :],
                                gT_sb[:, 0:w, :], gT_sb[:, s:M, :],
                            )
                        for t in range(off, off + w):
                            nc.tensor.matmul(
                                s_ps[0:NP2, :], sw_sb[:, t, :], h_sb[:, t, :],
                                start=(t == 0), stop=(t == NPAIR - 1),
                            )
                        off += w

                    # e = tanh(sqrt(s)) via single-table Ln/Exp path:
                    #   r = Exp(0.5*Ln(max(s,tiny))); v = Exp(2*r)
                    #   e = 1 - 2/(v+1)   (reciprocal approx on DVE)
                    s0 = wpool.tile([128, 128], BF16, tag="s0")
                    lns = wpool.tile([128, 128], BF16, tag="lns")
                    r_sb = wpool.tile([128, 128], BF16, tag="r")
                    v_sb = wpool.tile([128, 128], FP32, tag="v")
                    w_sb = wpool.tile([128, 128], FP32, tag="w")
                    rec = wpool.tile([128, 128], FP32, tag="rec")
                    e_sb = wpool.tile([128, 128], BF16, tag="e")
                    nc.vector.tensor_scalar_max(s0[0:NP2, :], s_ps[0:NP2, :], 1e-30)
                    nc.scalar.activation(lns[0:NP2, :], s0[0:NP2, :], AF.Ln)
                    nc.scalar.activation(
                        r_sb[0:NP2, :], lns[0:NP2, :], AF.Exp, scale=0.5
                    )
                    nc.scalar.activation(
                        v_sb[0:NP2, :], r_sb[0:NP2, :], AF.Exp, scale=2.0
                    )
                    nc.vector.tensor_scalar_add(w_sb[0:NP2, :], v_sb[0:NP2, :], 1.0)
                    nc.vector.reciprocal_approx_fast(rec[0:NP2, :], w_sb[0:NP2, :])
                    nc.vector.tensor_scalar(
                        e_sb[0:NP2, :], rec[0:NP2, :],
                        -2.0, 1.0,
                        bass.mybir.AluOpType.mult, bass.mybir.AluOpType.add,
                    )

                    # cT[n,k] = sum_p e[p,n] * minc[p,k]  (PE), fp32
                    ct_ps = ps_s.tile([128, 16], FP32, tag="ct")
                    nc.tensor.matmul(
                        ct_ps[0:128, 0:M], e_sb[0:NP2, :], minc_sb[0:NP2, 0:M]
                    )
                    cT_sb = wpool.tile([128, 16], FP32, tag="cT")
                    nc.scalar.activation(cT_sb[:, 0:M], ct_ps[0:128, 0:M], AF.Copy)

                    # combine: tk = c_k * g_k (DVE 4x), tree-sum on GPSIMD
                    tk = wpool.tile([128, M, E], BF16, tag="tk")
                    for k in range(M):
                        nc.vector.tensor_scalar_mul(
                            tk[:, k, :], g_sb[:, k, :], cT_sb[:, k:k + 1]
                        )
                    u4 = wpool.tile([128, 4, E], BF16, tag="u4")
                    u2 = wpool.tile([128, 2, E], BF16, tag="u2")
                    u1 = wpool.tile([128, E], BF16, tag="u1")
                    y1 = wpool.tile([128, E], BF16, tag="y1")
                    y_sb = wpool.tile([128, E], FP32, tag="y")
                    nc.vector.tensor_add(u4[:], tk[:, 0:4, :], tk[:, 4:8, :])
                    nc.vector.tensor_add(u2[:], u4[:, 0:2, :], u4[:, 2:4, :])
                    nc.vector.tensor_add(u1[:], u2[:, 0, :], u2[:, 1, :])
                    nc.vector.tensor_add(y1[:], u1[:], tk[:, 8, :])
                    nc.vector.tensor_add(y_sb[:], y1[:], hs[:])
                    nc.sync.dma_start(y_dr[nt * 128:(nt + 1) * 128, :], y_sb[:])

            for _ in range(reps):
                _body()


def _build_nc(reps=1):
    from concourse import bacc, mybir
    from concourse.hw_specs import get_activation_tables

    nc = bacc.Bacc(target_bir_lowering=False, debug=False)
    _emit(nc, reps=reps)

    # All activation funcs used (Relu/Ln/Exp/Copy) live in one table set;
    # replace the per-func first-match load pass (which thrashes between
    # sets) with a single load of that set at kernel start.
    tables = list(get_activation_tables(nc.m.arch))
    set_id = tables.index("natural_log_exp_and_others")
    funcs = {
        i.func
        for b in nc.main_func.blocks
        for i in b.instructions
        if isinstance(i, mybir.InstActivation)
    }
    allowed = get_activation_tables(nc.m.arch)["natural_log_exp_and_others"]
    assert funcs <= allowed | {mybir.ActivationFunctionType.Copy}, funcs

    def _single_act_load():
        inst = mybir.InstLoadActFuncSet(
            name=nc.get_next_instruction_name(),
            act_func_set_id=set_id,
            engine=mybir.EngineType.Activation,
        )
        nc.main_func.blocks[0].instructions.insert(0, inst)

    nc.insert_act_table_loads = _single_act_load
    nc.compile()
    return nc


def _prepare(f: np.ndarray, W: np.ndarray, reps=1):
    import ml_dtypes

    BF = ml_dtypes.bfloat16
    f = np.asarray(f, np.float32)
    wt = np.ascontiguousarray(
        np.transpose(np.asarray(W, np.float32), (2, 0, 1)).reshape(D, M * E).astype(BF)
    )
    sw, minc = _host_consts()
    base = {
        "wt": wt,
        "sw": np.ascontiguousarray(sw.astype(BF)),
        "minc": np.ascontiguousarray(minc.astype(BF)),
    }

    nc = _build_nc(reps=reps)
    in_maps = []
    for c in range(NCORES):
        fc = f[c * NLOC:(c + 1) * NLOC]                      # [1024, 9, 128]
        ft = np.transpose(fc.reshape(NT, 128, M, D), (3, 0, 2, 1))
        ft = np.ascontiguousarray(ft.reshape(D, NT * M * 128).astype(BF))
        fh = np.ascontiguousarray((0.5 * fc).reshape(NLOC, M * D).astype(BF))
        in_maps.append(dict(base, ft=ft, fh=fh))
    return nc, in_maps


def _run(f: np.ndarray, W: np.ndarray, trace: bool = False):
    from concourse.bass_utils import run_bass_kernel_spmd

    nc, in_maps = _prepare(f, W)
    res = run_bass_kernel_spmd(nc, in_maps, list(range(NCORES)), trace=trace)
    out = np.concatenate([np.asarray(r["y"]) for r in res.results], axis=0)
    return np.ascontiguousarray(out.astype(np.float32)), res


def kernel(f: np.ndarray, W: np.ndarray) -> np.ndarray:
    out, _ = _run(f, W, trace=False)
    return out


if __name__ == "__main__":
    rng = np.random.default_rng(0)
    f = rng.standard_normal((N, M, D), dtype=np.float32)
    W = rng.standard_normal((M, E, D), dtype=np.float32)
    y = kernel(f=f, W=W)
    print("kernel out", y.shape, y.dtype, float(np.abs(y).mean()))
